# revision 25
# baseline (speedup 1.0000x reference)
"""Trainium2 Bass kernel for nn_Decoder (batch-sharded RNN decoder).

Math (per batch element, T timesteps):
    e_t   = W_in @ x_t + b_in
    s_t   = relu(W_rec @ (s_{t-1} + e_t) + b_rec)
    h_t   = relu(W_out @ s_t + b_out)
    o_t   = W_out2 @ h_t + b_out2

Kernel strategy (8 cores, data-parallel over batch: 8 batch elements/core):
  - Fold weights on device:  W_comb = W_rec @ W_in,  b_comb = W_rec @ b_in + b_rec
    so  s_t = relu(W_rec @ s_{t-1} + E_t)  with  E = W_comb @ x^T + b_comb
    (one big GEMM, hoisted out of the recurrence).
  - Host marshals x to [IN, T*BL] (t-major) per core so the contraction dim
    lands on SBUF partitions with contiguous DMA.
  - Pipelined on device: E-GEMM chunk n overlaps recurrence steps of chunk n-1.
  - Epilogue (W_out / W_out2) runs as batched GEMMs over all T after the scan.
"""

import os
import sys
from contextlib import ExitStack

import numpy as np

for _p in ("/opt/trn_rl_repo", os.path.expanduser("~/.axon_site/_ro/trn_rl_repo")):
    if os.path.isdir(_p) and _p not in sys.path:
        sys.path.insert(0, _p)

# Problem constants (hardcoded per contract).
B, T, IN, S = 64, 512, 4096, 256
H, O = S // 2, 2
NCORES = 8
BL = B // NCORES          # batch per core
NT = T * BL               # GEMM columns per core (t-major: col = t*BL + b)
KI = IN // 128            # 32 contraction chunks
CW = 512                  # GEMM chunk width (columns)
NCH = NT // CW            # 16 chunks
TPC = CW // BL            # 32 timesteps per chunk

# Tuned dtype config (see test.py sweeps): fp16 x/weights for the big GEMM
# (fp32 accumulate), fp16 weights+state for the recurrence matmuls.
GEMM_DTYPE = "float16"   # "float32" | "float32r" | "float16"
REC_DTYPE = "float16"    # "float32" | "float16"


def build_nc(gemm_dtype=GEMM_DTYPE, rec_dtype=REC_DTYPE, final=True):
    import concourse.tile as tile
    from concourse import bacc, mybir
    from concourse.masks import make_identity

    f32 = mybir.dt.float32
    f32r = mybir.dt.float32r
    rdt = getattr(mybir.dt, rec_dtype)
    Relu = mybir.ActivationFunctionType.Relu

    nc = bacc.Bacc("TRN2")
    xdt = getattr(mybir.dt, gemm_dtype)
    xt = nc.declare_dram_parameter("xt", [IN, NT], xdt, isOutput=False)
    w_in = nc.declare_dram_parameter("w_in", [S, IN], f32, isOutput=False)
    b_in = nc.declare_dram_parameter("b_in", [S, 1], f32, isOutput=False)
    w_rec = nc.declare_dram_parameter("w_rec", [S, S], f32, isOutput=False)
    b_rec = nc.declare_dram_parameter("b_rec", [S, 1], f32, isOutput=False)
    w_out = nc.declare_dram_parameter("w_out", [H, S], f32, isOutput=False)
    b_out = nc.declare_dram_parameter("b_out", [H, 1], f32, isOutput=False)
    w_out2 = nc.declare_dram_parameter("w_out2", [O, H], f32, isOutput=False)
    b_out2 = nc.declare_dram_parameter("b_out2", [O, 1], f32, isOutput=False)
    out = nc.declare_dram_parameter("out", [BL, T, O], f32, isOutput=True)

    with tile.TileContext(nc) as tc, ExitStack() as ctx:
        consts = ctx.enter_context(tc.tile_pool(name="consts", bufs=1))
        wpool = ctx.enter_context(tc.tile_pool(name="wcomb", bufs=1))
        epool = ctx.enter_context(tc.tile_pool(name="estate", bufs=1))

        mm = nc.tensor.matmul

        # ---- Phase 0: load weights, transpose, fold ----
        with tc.tile_pool(name="pset", bufs=2, space="PSUM") as pset, \
             tc.tile_pool(name="winp", bufs=1) as winpool:
            ident = consts.tile([128, 128], f32)
            make_identity(nc, ident)

            sb_win = winpool.tile([128, 2, IN], f32)
            nc.sync.dma_start(out=sb_win, in_=w_in.rearrange("(c p) i -> p c i", p=128))
            sb_wrec = consts.tile([128, 2, S], f32)
            nc.sync.dma_start(out=sb_wrec, in_=w_rec.rearrange("(c p) j -> p c j", p=128))
            sb_bin = consts.tile([128, 2, 1], f32)
            nc.sync.dma_start(out=sb_bin, in_=b_in.rearrange("(c p) o -> p c o", p=128))
            sb_brec = consts.tile([128, 2, 1], f32)
            nc.sync.dma_start(out=sb_brec, in_=b_rec.rearrange("(c p) o -> p c o", p=128))
            sb_bout = consts.tile([128, 1], f32)
            nc.sync.dma_start(out=sb_bout, in_=b_out[:])
            sb_woutn = consts.tile([128, S], f32)
            nc.sync.dma_start(out=sb_woutn, in_=w_out[:])
            sb_w2n = consts.tile([O, H], f32)
            nc.sync.dma_start(out=sb_w2n, in_=w_out2[:])
            sb_b2pp = consts.tile([O, 1], f32)
            nc.sync.dma_start(out=sb_b2pp, in_=b_out2[:])

            # W_rec^T in [s_in, s_out] layout; f32 master + recurrence-dtype copy.
            sb_wrecT32 = consts.tile([128, 2, S], f32)
            sb_wrecTr = (consts.tile([128, 2, S], rdt, name="sb_wrecTr")
                         if rec_dtype != "float32" else sb_wrecT32)
            for ki in range(2):
                for mj in range(2):
                    ps = pset.tile([128, 128], f32, tag="pst")
                    nc.tensor.transpose(ps, sb_wrec[:, mj, ki * 128:(ki + 1) * 128], ident)
                    nc.vector.tensor_copy(sb_wrecT32[:, ki, mj * 128:(mj + 1) * 128], ps)
                    if rec_dtype != "float32":
                        nc.scalar.copy(sb_wrecTr[:, ki, mj * 128:(mj + 1) * 128], ps)

            # W_out^T [s, h] chunks (dtype matches recurrence state for the matmul).
            sb_woutT = consts.tile([128, 2, H], rdt)
            for ki in range(2):
                ps = pset.tile([128, 128], f32, tag="pst")
                nc.tensor.transpose(ps, sb_woutn[:, ki * 128:(ki + 1) * 128], ident)
                nc.vector.tensor_copy(sb_woutT[:, ki, :], ps)

            # W_out2^T [h, o] (stationary for the final projection)
            sb_w2T = consts.tile([128, O], f32)
            ps = pset.tile([128, O], f32, tag="pst")
            nc.tensor.transpose(ps, sb_w2n, ident[:O, :O])
            nc.vector.tensor_copy(sb_w2T, ps)

            # b_comb = W_rec @ b_in + b_rec
            sb_bcomb = consts.tile([128, 2, 1], f32)
            for sc in range(2):
                ps = pset.tile([128, 1], f32, tag="pst")
                mm(ps, lhsT=sb_wrecT32[:, 0, sc * 128:(sc + 1) * 128], rhs=sb_bin[:, 0, :], start=True, stop=False)
                mm(ps, lhsT=sb_wrecT32[:, 1, sc * 128:(sc + 1) * 128], rhs=sb_bin[:, 1, :], start=False, stop=True)
                nc.vector.tensor_add(sb_bcomb[:, sc, :], ps, sb_brec[:, sc, :])

            # W_combT[i, s] = (W_rec @ W_in)^T, 32 chunks of [128, S].
            wtiles = []
            for ic in range(KI):
                ps = pset.tile([128, S], f32, tag="pfold")
                mm(ps, lhsT=sb_win[:, 0, ic * 128:(ic + 1) * 128], rhs=sb_wrecT32[:, 0, :], start=True, stop=False)
                mm(ps, lhsT=sb_win[:, 1, ic * 128:(ic + 1) * 128], rhs=sb_wrecT32[:, 1, :], start=False, stop=True)
                wc = wpool.tile([128, S], xdt, tag=f"wc{ic}")
                nc.vector.tensor_copy(wc, ps)
                wtiles.append(wc)

        # State tiles: chunk n, half c -> [128, CW] (state after relu).
        # E itself lives in PSUM: the recurrence accumulates W_rec@s onto the
        # E-GEMM psum, and one DVE tensor_scalar does bias-add + relu -> SBUF.
        etiles = [[epool.tile([128, CW], rdt, tag=f"e{n}_{c}", name=f"e{n}_{c}")
                   for c in range(2)] for n in range(NCH)]
        xt_r = xt.rearrange("(a p) n -> p a n", p=128)
        add, amax = mybir.AluOpType.add, mybir.AluOpType.max

        # ---- Phase 1: pipelined E-GEMM + recurrence ----
        # The E-GEMM for chunk n+1 is interleaved one matmul per recurrence
        # step of chunk n: the PE stays dense (HAM stays warm) and the GEMM
        # hides inside the recurrence's dependency stalls.
        with tc.tile_pool(name="pg", bufs=2, space="PSUM") as pg, \
             tc.tile_pool(name="xin", bufs=4) as xpool:
            gps, xts = {}, {}

            def emit_dmas(n):
                ts = []
                for h in range(2):
                    xt_t = xpool.tile([128, KI // 2, CW], xdt, tag="x", name=f"x{n}_{h}")
                    for q in range(4):
                        a0 = h * (KI // 2) + q * 4
                        nc.sync.dma_start(
                            out=xt_t[:, q * 4:(q + 1) * 4, :],
                            in_=xt_r[:, a0:a0 + 4, n * CW:(n + 1) * CW])
                    ts.append(xt_t)
                xts[n] = ts

            def gemm_mm_thunks(n):
                ps = [pg.tile([128, CW], f32, tag=f"g{c}", name=f"g{n}_{c}")
                      for c in range(2)]
                gps[n] = ps
                thunks = []
                for h in range(2):
                    for c in range(2):
                        for k in range(KI // 2):
                            ic = h * (KI // 2) + k
                            thunks.append(lambda h=h, c=c, k=k, ic=ic: mm(
                                ps[c], lhsT=wtiles[ic][:, c * 128:(c + 1) * 128],
                                rhs=xts[n][h][:, k, :],
                                start=(ic == 0), stop=(ic == KI - 1)))
                return thunks

            def relu_step(ps, n, j):
                # c0 on DVE, c1 on ACT -- the two halves run concurrently.
                sl = slice(j * BL, (j + 1) * BL)
                nc.vector.tensor_scalar(
                    out=etiles[n][0][:, sl], in0=ps[0][:, sl],
                    scalar1=sb_bcomb[:, 0, :], scalar2=0.0,
                    op0=mybir.AluOpType.add, op1=mybir.AluOpType.max)
                nc.scalar.activation(
                    etiles[n][1][:, sl], ps[1][:, sl], Relu,
                    bias=sb_bcomb[:, 1, :], scale=1.0)

            def recur_block(n, fillers):
                ps = gps.pop(n)
                for j in range(TPC):
                    t = n * TPC + j
                    if t > 0:
                        pn, pj = (t - 1) // TPC, (t - 1) % TPC
                        prev = [etiles[pn][kc][:, pj * BL:(pj + 1) * BL]
                                for kc in range(2)]
                        sl = slice(j * BL, (j + 1) * BL)
                        # k0 matmuls first: they only need half 0's relu.
                        for kc in range(2):
                            for c in range(2):
                                mm(ps[c][:, sl],
                                   lhsT=sb_wrecTr[:, kc, c * 128:(c + 1) * 128],
                                   rhs=prev[kc], start=False, stop=(kc == 1),
                                   skip_group_check=True)
                    for th in (next(fillers, None),):
                        if th:
                            th()
                    relu_step(ps, n, j)

            emit_dmas(0)
            for th in gemm_mm_thunks(0):
                th()
            for n in range(1, NCH):
                emit_dmas(n)
                recur_block(n - 1, iter(gemm_mm_thunks(n)))
            recur_block(NCH - 1, iter(()))

        # ---- Phase 2: epilogue h/o GEMMs over all T ----
        with tc.tile_pool(name="pe", bufs=2, space="PSUM") as pe, \
             tc.tile_pool(name="po", bufs=4, space="PSUM") as po, \
             tc.tile_pool(name="hbuf", bufs=1) as hpool:
            sb_hh = hpool.tile([128, NT], f32)
            for n in range(NCH):
                ps = pe.tile([128, CW], f32, tag="ph")
                mm(ps, lhsT=sb_woutT[:, 0, :], rhs=etiles[n][0], start=True, stop=False)
                mm(ps, lhsT=sb_woutT[:, 1, :], rhs=etiles[n][1], start=False, stop=True)
                nc.scalar.activation(sb_hh[:, n * CW:(n + 1) * CW], ps, Relu, bias=sb_bout, scale=1.0)

            # Final projection: psum [O, 512] -> bias add -> SBUF -> DMA out.
            # Free dim is col = t*BL + b -> out[b, t, o].
            sb_o = hpool.tile([O, NT], f32)
            for q in range(NT // 512):
                ps = po.tile([O, 512], f32, tag="po")
                mm(ps, lhsT=sb_w2T, rhs=sb_hh[:, q * 512:(q + 1) * 512], start=True, stop=True)
                nc.vector.tensor_scalar_add(sb_o[:, q * 512:(q + 1) * 512], ps, sb_b2pp)
            out_otb = out.rearrange("b t o -> o t b")
            sb_o_r = sb_o.rearrange("o (t b) -> o t b", b=BL)
            for oo in range(O):
                nc.sync.dma_start(out=out_otb[oo:oo + 1], in_=sb_o_r[oo:oo + 1])

    if final:
        nc.compile()
    return nc


def _in_maps(x, W_in, b_in, W_rec, b_rec, W_out, b_out, W_out2, b_out2,
             gemm_dtype=GEMM_DTYPE):
    xnp = np.float16 if gemm_dtype == "float16" else np.float32
    maps = []
    for c in range(NCORES):
        xb = np.ascontiguousarray(
            x[c * BL:(c + 1) * BL].transpose(2, 1, 0).reshape(IN, NT).astype(xnp))
        maps.append({
            "xt": xb,
            "w_in": np.ascontiguousarray(W_in),
            "b_in": np.ascontiguousarray(b_in.reshape(S, 1)),
            "w_rec": np.ascontiguousarray(W_rec),
            "b_rec": np.ascontiguousarray(b_rec.reshape(S, 1)),
            "w_out": np.ascontiguousarray(W_out),
            "b_out": np.ascontiguousarray(b_out.reshape(H, 1)),
            "w_out2": np.ascontiguousarray(W_out2),
            "b_out2": np.ascontiguousarray(b_out2.reshape(O, 1)),
        })
    return maps


def kernel(x, W_in, b_in, W_rec, b_rec, W_out, b_out, W_out2, b_out2):
    from concourse.bass_utils import run_bass_kernel_spmd

    args = [np.asarray(a, dtype=np.float32) for a in
            (x, W_in, b_in, W_rec, b_rec, W_out, b_out, W_out2, b_out2)]
    nc = build_nc()
    res = run_bass_kernel_spmd(nc, _in_maps(*args), list(range(NCORES))).results
    out = np.empty((B, T, O), np.float32)
    for c in range(NCORES):
        out[c * BL:(c + 1) * BL] = res[c]["out"]
    return out


if __name__ == "__main__":
    rng = np.random.default_rng(0)
    ins = {
        "x": rng.standard_normal((B, T, IN), dtype=np.float32),
        "W_in": rng.standard_normal((S, IN), dtype=np.float32) * 0.01,
        "b_in": rng.standard_normal((S,), dtype=np.float32) * 0.01,
        "W_rec": rng.standard_normal((S, S), dtype=np.float32) * 0.05,
        "b_rec": rng.standard_normal((S,), dtype=np.float32) * 0.05,
        "W_out": rng.standard_normal((H, S), dtype=np.float32) * 0.05,
        "b_out": rng.standard_normal((H,), dtype=np.float32) * 0.05,
        "W_out2": rng.standard_normal((O, H), dtype=np.float32) * 0.05,
        "b_out2": rng.standard_normal((O,), dtype=np.float32) * 0.05,
    }
    print(kernel(**ins).shape)


# revision 28
# speedup vs baseline: 1.0248x; 1.0248x over previous
"""Trainium2 Bass kernel for nn_Decoder (batch-sharded RNN decoder).

Math (per batch element, T timesteps):
    e_t   = W_in @ x_t + b_in
    s_t   = relu(W_rec @ (s_{t-1} + e_t) + b_rec)
    h_t   = relu(W_out @ s_t + b_out)
    o_t   = W_out2 @ h_t + b_out2

Kernel strategy (8 cores, data-parallel over batch: 8 batch elements/core):
  - Fold weights on device:  W_comb = W_rec @ W_in,  b_comb = W_rec @ b_in + b_rec
    so  s_t = relu(W_rec @ s_{t-1} + E_t)  with  E = W_comb @ x^T + b_comb
    (one big GEMM, hoisted out of the recurrence).
  - Host marshals x to [IN, T*BL] (t-major) per core so the contraction dim
    lands on SBUF partitions with contiguous DMA.
  - Pipelined on device: E-GEMM chunk n overlaps recurrence steps of chunk n-1.
  - Epilogue (W_out / W_out2) runs as batched GEMMs over all T after the scan.
"""

import os
import sys
from contextlib import ExitStack

import numpy as np

for _p in ("/opt/trn_rl_repo", os.path.expanduser("~/.axon_site/_ro/trn_rl_repo")):
    if os.path.isdir(_p) and _p not in sys.path:
        sys.path.insert(0, _p)

# Problem constants (hardcoded per contract).
B, T, IN, S = 64, 512, 4096, 256
H, O = S // 2, 2
NCORES = 8
BL = B // NCORES          # batch per core
NT = T * BL               # GEMM columns per core (t-major: col = t*BL + b)
KI = IN // 128            # 32 contraction chunks
CW = 512                  # GEMM chunk width (columns)
NCH = NT // CW            # 16 chunks
TPC = CW // BL            # 32 timesteps per chunk

# Tuned dtype config (see test.py sweeps): fp16 x/weights for the big GEMM
# (fp32 accumulate), fp16 weights+state for the recurrence matmuls.
GEMM_DTYPE = "float16"   # "float32" | "float32r" | "float16"
REC_DTYPE = "float16"    # "float32" | "float16" | "mixed" (fp16 W, fp32 state)


def build_nc(gemm_dtype=GEMM_DTYPE, rec_dtype=REC_DTYPE, final=True):
    import concourse.tile as tile
    from concourse import bacc, mybir
    from concourse.masks import make_identity

    f32 = mybir.dt.float32
    f32r = mybir.dt.float32r
    wdt = mybir.dt.float16 if rec_dtype == "mixed" else getattr(mybir.dt, rec_dtype)
    edt = mybir.dt.float32r if rec_dtype == "mixed" else getattr(mybir.dt, rec_dtype)
    Relu = mybir.ActivationFunctionType.Relu

    nc = bacc.Bacc("TRN2")
    xdt = getattr(mybir.dt, gemm_dtype)
    xt = nc.declare_dram_parameter("xt", [IN, NT], xdt, isOutput=False)
    w_in = nc.declare_dram_parameter("w_in", [S, IN], f32, isOutput=False)
    b_in = nc.declare_dram_parameter("b_in", [S, 1], f32, isOutput=False)
    w_rec = nc.declare_dram_parameter("w_rec", [S, S], f32, isOutput=False)
    b_rec = nc.declare_dram_parameter("b_rec", [S, 1], f32, isOutput=False)
    w_out = nc.declare_dram_parameter("w_out", [H, S], f32, isOutput=False)
    b_out = nc.declare_dram_parameter("b_out", [H, 1], f32, isOutput=False)
    w_out2 = nc.declare_dram_parameter("w_out2", [O, H], f32, isOutput=False)
    b_out2 = nc.declare_dram_parameter("b_out2", [O, 1], f32, isOutput=False)
    out = nc.declare_dram_parameter("out", [BL, T, O], f32, isOutput=True)

    with tile.TileContext(nc) as tc, ExitStack() as ctx:
        consts = ctx.enter_context(tc.tile_pool(name="consts", bufs=1))
        wpool = ctx.enter_context(tc.tile_pool(name="wcomb", bufs=1))
        epool = ctx.enter_context(tc.tile_pool(name="estate", bufs=1))

        mm = nc.tensor.matmul

        # ---- Phase 0: load weights, transpose, fold ----
        with tc.tile_pool(name="pset", bufs=2, space="PSUM") as pset, \
             tc.tile_pool(name="winp", bufs=1) as winpool:
            ident = consts.tile([128, 128], f32)
            make_identity(nc, ident)

            sb_win = winpool.tile([128, 2, IN], f32)
            nc.sync.dma_start(out=sb_win, in_=w_in.rearrange("(c p) i -> p c i", p=128))
            sb_wrec = consts.tile([128, 2, S], f32)
            nc.sync.dma_start(out=sb_wrec, in_=w_rec.rearrange("(c p) j -> p c j", p=128))
            sb_bin = consts.tile([128, 2, 1], f32)
            nc.sync.dma_start(out=sb_bin, in_=b_in.rearrange("(c p) o -> p c o", p=128))
            sb_brec = consts.tile([128, 2, 1], f32)
            nc.sync.dma_start(out=sb_brec, in_=b_rec.rearrange("(c p) o -> p c o", p=128))
            sb_bout = consts.tile([128, 1], f32)
            nc.sync.dma_start(out=sb_bout, in_=b_out[:])
            sb_woutn = consts.tile([128, S], f32)
            nc.sync.dma_start(out=sb_woutn, in_=w_out[:])
            sb_w2n = consts.tile([O, H], f32)
            nc.sync.dma_start(out=sb_w2n, in_=w_out2[:])
            sb_b2pp = consts.tile([O, 1], f32)
            nc.sync.dma_start(out=sb_b2pp, in_=b_out2[:])

            # W_rec^T in [s_in, s_out] layout; f32 master + recurrence-dtype copy.
            sb_wrecT32 = consts.tile([128, 2, S], f32)
            sb_wrecTr = (consts.tile([128, 2, S], wdt, name="sb_wrecTr")
                         if rec_dtype != "float32" else sb_wrecT32)
            for ki in range(2):
                for mj in range(2):
                    ps = pset.tile([128, 128], f32, tag="pst")
                    nc.tensor.transpose(ps, sb_wrec[:, mj, ki * 128:(ki + 1) * 128], ident)
                    nc.vector.tensor_copy(sb_wrecT32[:, ki, mj * 128:(mj + 1) * 128], ps)
                    if rec_dtype != "float32":
                        nc.scalar.copy(sb_wrecTr[:, ki, mj * 128:(mj + 1) * 128], ps)

            # W_out^T [s, h] chunks (dtype matches recurrence state for the matmul).
            sb_woutT = consts.tile([128, 2, H], edt)
            for ki in range(2):
                ps = pset.tile([128, 128], f32, tag="pst")
                nc.tensor.transpose(ps, sb_woutn[:, ki * 128:(ki + 1) * 128], ident)
                nc.vector.tensor_copy(sb_woutT[:, ki, :], ps)

            # W_out2^T [h, o] (stationary for the final projection)
            sb_w2T = consts.tile([128, O], f32)
            ps = pset.tile([128, O], f32, tag="pst")
            nc.tensor.transpose(ps, sb_w2n, ident[:O, :O])
            nc.vector.tensor_copy(sb_w2T, ps)

            # b_comb = W_rec @ b_in + b_rec
            sb_bcomb = consts.tile([128, 2, 1], f32)
            for sc in range(2):
                ps = pset.tile([128, 1], f32, tag="pst")
                mm(ps, lhsT=sb_wrecT32[:, 0, sc * 128:(sc + 1) * 128], rhs=sb_bin[:, 0, :], start=True, stop=False)
                mm(ps, lhsT=sb_wrecT32[:, 1, sc * 128:(sc + 1) * 128], rhs=sb_bin[:, 1, :], start=False, stop=True)
                nc.vector.tensor_add(sb_bcomb[:, sc, :], ps, sb_brec[:, sc, :])

            # W_combT[i, s] = (W_rec @ W_in)^T, 32 chunks of [128, S].
            wtiles = []
            for ic in range(KI):
                ps = pset.tile([128, S], f32, tag="pfold")
                mm(ps, lhsT=sb_win[:, 0, ic * 128:(ic + 1) * 128], rhs=sb_wrecT32[:, 0, :], start=True, stop=False)
                mm(ps, lhsT=sb_win[:, 1, ic * 128:(ic + 1) * 128], rhs=sb_wrecT32[:, 1, :], start=False, stop=True)
                wc = wpool.tile([128, S], xdt, tag=f"wc{ic}")
                nc.vector.tensor_copy(wc, ps)
                wtiles.append(wc)

        # State tiles: chunk n, half c -> [128, CW] (state after relu).
        # E itself lives in PSUM: the recurrence accumulates W_rec@s onto the
        # E-GEMM psum, and one DVE tensor_scalar does bias-add + relu -> SBUF.
        etiles = [[epool.tile([128, CW], edt, tag=f"e{n}_{c}", name=f"e{n}_{c}")
                   for c in range(2)] for n in range(NCH)]
        xt_r = xt.rearrange("(a p) n -> p a n", p=128)
        add, amax = mybir.AluOpType.add, mybir.AluOpType.max

        # ---- Phase 1+2: pipelined E-GEMM + recurrence + interleaved epilogue ----
        # Chunk n+1's E-GEMM matmuls and chunk n-1's epilogue (h/o projections,
        # output DMA) are interleaved into chunk n's recurrence steps: the PE
        # stays dense (HAM warm) and all non-recurrent work hides inside the
        # recurrence's dependency stalls.
        xbufs = 4 if gemm_dtype == "float16" else 2
        with tc.tile_pool(name="pg", bufs=2, space="PSUM") as pg, \
             tc.tile_pool(name="pe", bufs=2, space="PSUM") as pe, \
             tc.tile_pool(name="po", bufs=2, space="PSUM") as po, \
             tc.tile_pool(name="xin", bufs=xbufs) as xpool, \
             tc.tile_pool(name="hbuf", bufs=1) as hpool:
            sb_hh = hpool.tile([128, NT], f32)
            sb_o = hpool.tile([O, NT], f32)
            out_otb = out.rearrange("b t o -> o t b")
            sb_o_r = sb_o.rearrange("o (t b) -> o t b", b=BL)
            gps, xts = {}, {}

            def emit_dmas(n):
                ts = []
                for h in range(2):
                    xt_t = xpool.tile([128, KI // 2, CW], xdt, tag="x", name=f"x{n}_{h}")
                    for q in range(4):
                        a0 = h * (KI // 2) + q * 4
                        nc.sync.dma_start(
                            out=xt_t[:, q * 4:(q + 1) * 4, :],
                            in_=xt_r[:, a0:a0 + 4, n * CW:(n + 1) * CW])
                    ts.append(xt_t)
                xts[n] = ts

            def gemm_mm_thunks(n):
                ps = [pg.tile([128, CW], f32, tag=f"g{c}", name=f"g{n}_{c}")
                      for c in range(2)]
                gps[n] = ps
                thunks = []
                for h in range(2):
                    for c in range(2):
                        for k in range(KI // 2):
                            ic = h * (KI // 2) + k
                            thunks.append(lambda h=h, c=c, k=k, ic=ic: mm(
                                ps[c], lhsT=wtiles[ic][:, c * 128:(c + 1) * 128],
                                rhs=xts[n][h][:, k, :],
                                start=(ic == 0), stop=(ic == KI - 1)))
                return thunks

            def epi_thunks(n):
                sl = slice(n * CW, (n + 1) * CW)
                tsl = slice(n * TPC, (n + 1) * TPC)
                ps_h = pe.tile([128, CW], f32, tag="ph", name=f"ph{n}")
                ps_o = po.tile([O, CW], f32, tag="po", name=f"po{n}")
                return [
                    lambda: mm(ps_h, lhsT=sb_woutT[:, 0, :], rhs=etiles[n][0],
                               start=True, stop=False),
                    lambda: mm(ps_h, lhsT=sb_woutT[:, 1, :], rhs=etiles[n][1],
                               start=False, stop=True),
                    lambda: nc.scalar.activation(sb_hh[:, sl], ps_h, Relu,
                                                 bias=sb_bout, scale=1.0),
                    lambda: mm(ps_o, lhsT=sb_w2T, rhs=sb_hh[:, sl],
                               start=True, stop=True),
                    lambda: nc.vector.tensor_scalar_add(sb_o[:, sl], ps_o, sb_b2pp),
                ] + [
                    (lambda oo=oo: nc.sync.dma_start(
                        out=out_otb[oo:oo + 1, tsl, :],
                        in_=sb_o_r[oo:oo + 1, tsl, :])) for oo in range(O)
                ]

            def relu_step(ps, n, j):
                # c0 on DVE, c1 on ACT -- the two halves run concurrently.
                sl = slice(j * BL, (j + 1) * BL)
                nc.vector.tensor_scalar(
                    out=etiles[n][0][:, sl], in0=ps[0][:, sl],
                    scalar1=sb_bcomb[:, 0, :], scalar2=0.0,
                    op0=mybir.AluOpType.add, op1=mybir.AluOpType.max)
                nc.scalar.activation(
                    etiles[n][1][:, sl], ps[1][:, sl], Relu,
                    bias=sb_bcomb[:, 1, :], scale=1.0)

            def recur_block(n, fillers, fillers2=()):
                ps = gps.pop(n)
                f2 = iter(fillers2)
                for j in range(TPC):
                    t = n * TPC + j
                    if t > 0:
                        pn, pj = (t - 1) // TPC, (t - 1) % TPC
                        prev = [etiles[pn][kc][:, pj * BL:(pj + 1) * BL]
                                for kc in range(2)]
                        sl = slice(j * BL, (j + 1) * BL)
                        # k0 matmuls first: they only need half 0's relu.
                        for kc in range(2):
                            for c in range(2):
                                mm(ps[c][:, sl],
                                   lhsT=sb_wrecTr[:, kc, c * 128:(c + 1) * 128],
                                   rhs=prev[kc], start=False, stop=(kc == 1),
                                   skip_group_check=True)
                    th = next(fillers, None)
                    if th:
                        th()
                    if j % 7 == 3:
                        th2 = next(f2, None)
                        if th2:
                            th2()
                    relu_step(ps, n, j)

            emit_dmas(0)
            for th in gemm_mm_thunks(0):
                th()
            for n in range(1, NCH):
                emit_dmas(n)
                recur_block(n - 1, iter(gemm_mm_thunks(n)),
                            epi_thunks(n - 2) if n >= 2 else ())
            recur_block(NCH - 1, iter(()), epi_thunks(NCH - 2))
            for th in epi_thunks(NCH - 1):
                th()

    if final:
        nc.compile()
    return nc


def _in_maps(x, W_in, b_in, W_rec, b_rec, W_out, b_out, W_out2, b_out2,
             gemm_dtype=GEMM_DTYPE):
    xnp = np.float16 if gemm_dtype == "float16" else np.float32
    maps = []
    for c in range(NCORES):
        xb = np.ascontiguousarray(
            x[c * BL:(c + 1) * BL].transpose(2, 1, 0).reshape(IN, NT).astype(xnp))
        maps.append({
            "xt": xb,
            "w_in": np.ascontiguousarray(W_in),
            "b_in": np.ascontiguousarray(b_in.reshape(S, 1)),
            "w_rec": np.ascontiguousarray(W_rec),
            "b_rec": np.ascontiguousarray(b_rec.reshape(S, 1)),
            "w_out": np.ascontiguousarray(W_out),
            "b_out": np.ascontiguousarray(b_out.reshape(H, 1)),
            "w_out2": np.ascontiguousarray(W_out2),
            "b_out2": np.ascontiguousarray(b_out2.reshape(O, 1)),
        })
    return maps


def kernel(x, W_in, b_in, W_rec, b_rec, W_out, b_out, W_out2, b_out2):
    from concourse.bass_utils import run_bass_kernel_spmd

    args = [np.asarray(a, dtype=np.float32) for a in
            (x, W_in, b_in, W_rec, b_rec, W_out, b_out, W_out2, b_out2)]
    nc = build_nc()
    res = run_bass_kernel_spmd(nc, _in_maps(*args), list(range(NCORES))).results
    out = np.empty((B, T, O), np.float32)
    for c in range(NCORES):
        out[c * BL:(c + 1) * BL] = res[c]["out"]
    return out


if __name__ == "__main__":
    rng = np.random.default_rng(0)
    ins = {
        "x": rng.standard_normal((B, T, IN), dtype=np.float32),
        "W_in": rng.standard_normal((S, IN), dtype=np.float32) * 0.01,
        "b_in": rng.standard_normal((S,), dtype=np.float32) * 0.01,
        "W_rec": rng.standard_normal((S, S), dtype=np.float32) * 0.05,
        "b_rec": rng.standard_normal((S,), dtype=np.float32) * 0.05,
        "W_out": rng.standard_normal((H, S), dtype=np.float32) * 0.05,
        "b_out": rng.standard_normal((H,), dtype=np.float32) * 0.05,
        "W_out2": rng.standard_normal((O, H), dtype=np.float32) * 0.05,
        "b_out2": rng.standard_normal((O,), dtype=np.float32) * 0.05,
    }
    print(kernel(**ins).shape)


# revision 32
# speedup vs baseline: 1.3874x; 1.3539x over previous
"""Trainium2 Bass kernel for nn_Decoder (batch-sharded RNN decoder).

Math (per batch element, T timesteps):
    e_t   = W_in @ x_t + b_in
    s_t   = relu(W_rec @ (s_{t-1} + e_t) + b_rec)
    h_t   = relu(W_out @ s_t + b_out)
    o_t   = W_out2 @ h_t + b_out2

Kernel strategy (8 cores, data-parallel over batch: 8 batch elements/core):
  - Fold weights on device:  W_comb = W_rec @ W_in,  b_comb = W_rec @ b_in + b_rec
    so  s_t = relu(W_rec @ s_{t-1} + E_t)  with  E = W_comb @ x^T + b_comb
    (one big GEMM, hoisted out of the recurrence).
  - Host marshals x to [IN, T*BL] (t-major) per core so the contraction dim
    lands on SBUF partitions with contiguous DMA.
  - Pipelined on device: E-GEMM chunk n overlaps recurrence steps of chunk n-1.
  - Epilogue (W_out / W_out2) runs as batched GEMMs over all T after the scan.
"""

import os
import sys
from contextlib import ExitStack

import numpy as np

for _p in ("/opt/trn_rl_repo", os.path.expanduser("~/.axon_site/_ro/trn_rl_repo")):
    if os.path.isdir(_p) and _p not in sys.path:
        sys.path.insert(0, _p)

# Problem constants (hardcoded per contract).
B, T, IN, S = 64, 512, 4096, 256
H, O = S // 2, 2
NCORES = 8
BL = B // NCORES          # batch per core
NT = T * BL               # GEMM columns per core (t-major: col = t*BL + b)
KI = IN // 128            # 32 contraction chunks
CW = 512                  # GEMM chunk width (columns)
NCH = NT // CW            # 16 chunks
TPC = CW // BL            # 32 timesteps per chunk

# Tuned dtype config (see test.py sweeps): fp16 x/weights for the big GEMM
# (fp32 accumulate), fp16 weights+state for the recurrence matmuls.
GEMM_DTYPE = "float16"   # "float32" | "float32r" | "float16"
REC_DTYPE = "float16"    # "float32" | "float16" | "mixed" (fp16 W, fp32 state)


def build_nc(gemm_dtype=GEMM_DTYPE, rec_dtype=REC_DTYPE, final=True):
    import concourse.tile as tile
    from concourse import bacc, mybir
    from concourse.masks import make_identity

    f32 = mybir.dt.float32
    f32r = mybir.dt.float32r
    wdt = mybir.dt.float16 if rec_dtype == "mixed" else getattr(mybir.dt, rec_dtype)
    edt = mybir.dt.float32r if rec_dtype == "mixed" else getattr(mybir.dt, rec_dtype)
    Relu = mybir.ActivationFunctionType.Relu

    nc = bacc.Bacc("TRN2")
    xdt = getattr(mybir.dt, gemm_dtype)
    xt = nc.declare_dram_parameter("xt", [IN, NT], xdt, isOutput=False)
    w_in = nc.declare_dram_parameter("w_in", [S, IN], f32, isOutput=False)
    b_in = nc.declare_dram_parameter("b_in", [S, 1], f32, isOutput=False)
    w_rec = nc.declare_dram_parameter("w_rec", [S, S], f32, isOutput=False)
    b_rec = nc.declare_dram_parameter("b_rec", [S, 1], f32, isOutput=False)
    w_out = nc.declare_dram_parameter("w_out", [H, S], f32, isOutput=False)
    b_out = nc.declare_dram_parameter("b_out", [H, 1], f32, isOutput=False)
    w_out2 = nc.declare_dram_parameter("w_out2", [O, H], f32, isOutput=False)
    b_out2 = nc.declare_dram_parameter("b_out2", [O, 1], f32, isOutput=False)
    out = nc.declare_dram_parameter("out", [BL, T, O], f32, isOutput=True)

    with tile.TileContext(nc) as tc, ExitStack() as ctx:
        consts = ctx.enter_context(tc.tile_pool(name="consts", bufs=1))
        wpool = ctx.enter_context(tc.tile_pool(name="wcomb", bufs=1))
        epool = ctx.enter_context(tc.tile_pool(name="estate", bufs=1))

        mm = nc.tensor.matmul

        # ---- Phase 0: load weights, transpose, fold ----
        with tc.tile_pool(name="pset", bufs=2, space="PSUM") as pset, \
             tc.tile_pool(name="winp", bufs=1) as winpool:
            ident = consts.tile([128, 128], f32)
            make_identity(nc, ident)

            sb_win = winpool.tile([128, 2, IN], f32)
            nc.sync.dma_start(out=sb_win, in_=w_in.rearrange("(c p) i -> p c i", p=128))
            sb_wrec = consts.tile([128, 2, S], f32)
            nc.sync.dma_start(out=sb_wrec, in_=w_rec.rearrange("(c p) j -> p c j", p=128))
            sb_bin = consts.tile([128, 2, 1], f32)
            nc.sync.dma_start(out=sb_bin, in_=b_in.rearrange("(c p) o -> p c o", p=128))
            sb_brec = consts.tile([128, 2, 1], f32)
            nc.sync.dma_start(out=sb_brec, in_=b_rec.rearrange("(c p) o -> p c o", p=128))
            sb_bout = consts.tile([128, 1], f32)
            nc.sync.dma_start(out=sb_bout, in_=b_out[:])
            sb_woutn = consts.tile([128, S], f32)
            nc.sync.dma_start(out=sb_woutn, in_=w_out[:])
            sb_w2n = consts.tile([O, H], f32)
            nc.sync.dma_start(out=sb_w2n, in_=w_out2[:])
            sb_b2pp = consts.tile([O, 1], f32)
            nc.sync.dma_start(out=sb_b2pp, in_=b_out2[:])

            # W_rec^T in [s_in, s_out] layout; f32 master + recurrence-dtype copy.
            sb_wrecT32 = consts.tile([128, 2, S], f32)
            sb_wrecTr = (consts.tile([128, 2, S], wdt, name="sb_wrecTr")
                         if rec_dtype != "float32" else sb_wrecT32)
            for ki in range(2):
                for mj in range(2):
                    ps = pset.tile([128, 128], f32, tag="pst")
                    nc.tensor.transpose(ps, sb_wrec[:, mj, ki * 128:(ki + 1) * 128], ident)
                    nc.vector.tensor_copy(sb_wrecT32[:, ki, mj * 128:(mj + 1) * 128], ps)
                    if rec_dtype != "float32":
                        nc.scalar.copy(sb_wrecTr[:, ki, mj * 128:(mj + 1) * 128], ps)

            # W_out^T [s, h] chunks (dtype matches recurrence state for the matmul).
            sb_woutT = consts.tile([128, 2, H], edt)
            for ki in range(2):
                ps = pset.tile([128, 128], f32, tag="pst")
                nc.tensor.transpose(ps, sb_woutn[:, ki * 128:(ki + 1) * 128], ident)
                nc.vector.tensor_copy(sb_woutT[:, ki, :], ps)

            # W_out2^T [h, o] (stationary for the final projection)
            sb_w2T = consts.tile([128, O], f32)
            ps = pset.tile([128, O], f32, tag="pst")
            nc.tensor.transpose(ps, sb_w2n, ident[:O, :O])
            nc.vector.tensor_copy(sb_w2T, ps)

            # b_comb = W_rec @ b_in + b_rec
            sb_bcomb = consts.tile([128, 2, 1], f32)
            for sc in range(2):
                ps = pset.tile([128, 1], f32, tag="pst")
                mm(ps, lhsT=sb_wrecT32[:, 0, sc * 128:(sc + 1) * 128], rhs=sb_bin[:, 0, :], start=True, stop=False)
                mm(ps, lhsT=sb_wrecT32[:, 1, sc * 128:(sc + 1) * 128], rhs=sb_bin[:, 1, :], start=False, stop=True)
                nc.vector.tensor_add(sb_bcomb[:, sc, :], ps, sb_brec[:, sc, :])

            # Bias as a K=1 matmul operand: b_combT [1, 2, 128] + ones [1, CW].
            sb_bcombT = consts.tile([1, 2, 128], xdt)
            for sc in range(2):
                ps = pset.tile([1, 128], f32, tag="pbt", name=f"pbt{sc}")
                nc.tensor.transpose(ps, sb_bcomb[:, sc, :], ident)
                nc.vector.tensor_copy(sb_bcombT[:, sc, :], ps)
            sb_ones = consts.tile([1, CW], xdt)
            nc.vector.memset(sb_ones, 1.0)

            # W_combT[i, s] = (W_rec @ W_in)^T, 32 chunks of [128, S].
            wtiles = []
            for ic in range(KI):
                ps = pset.tile([128, S], f32, tag="pfold")
                mm(ps, lhsT=sb_win[:, 0, ic * 128:(ic + 1) * 128], rhs=sb_wrecT32[:, 0, :], start=True, stop=False)
                mm(ps, lhsT=sb_win[:, 1, ic * 128:(ic + 1) * 128], rhs=sb_wrecT32[:, 1, :], start=False, stop=True)
                wc = wpool.tile([128, S], xdt, tag=f"wc{ic}")
                nc.vector.tensor_copy(wc, ps)
                wtiles.append(wc)

        # State tiles: chunk n, half c -> [128, CW] (state after relu).
        # E itself lives in PSUM: the recurrence accumulates W_rec@s onto the
        # E-GEMM psum, and one DVE tensor_scalar does bias-add + relu -> SBUF.
        etiles = [epool.tile([128, 2, CW], edt, tag=f"e{n}", name=f"e{n}")
                  for n in range(NCH)]
        xt_r = xt.rearrange("(a p) n -> p a n", p=128)
        add, amax = mybir.AluOpType.add, mybir.AluOpType.max

        # ---- Phase 1+2: pipelined E-GEMM + recurrence + interleaved epilogue ----
        # Chunk n+1's E-GEMM matmuls and chunk n-1's epilogue (h/o projections,
        # output DMA) are interleaved into chunk n's recurrence steps: the PE
        # stays dense (HAM warm) and all non-recurrent work hides inside the
        # recurrence's dependency stalls.
        xbufs = 4 if gemm_dtype == "float16" else 2
        with tc.tile_pool(name="pg", bufs=2, space="PSUM") as pg, \
             tc.tile_pool(name="pe", bufs=2, space="PSUM") as pe, \
             tc.tile_pool(name="po", bufs=2, space="PSUM") as po, \
             tc.tile_pool(name="xin", bufs=xbufs) as xpool, \
             tc.tile_pool(name="hbuf", bufs=1) as hpool:
            sb_hh = hpool.tile([128, NT], f32)
            sb_o = hpool.tile([O, NT], f32)
            out_otb = out.rearrange("b t o -> o t b")
            sb_o_r = sb_o.rearrange("o (t b) -> o t b", b=BL)
            gps, xts = {}, {}

            def emit_dmas(n):
                ts = []
                for h in range(2):
                    xt_t = xpool.tile([128, KI // 2, CW], xdt, tag="x", name=f"x{n}_{h}")
                    for q in range(4):
                        a0 = h * (KI // 2) + q * 4
                        nc.sync.dma_start(
                            out=xt_t[:, q * 4:(q + 1) * 4, :],
                            in_=xt_r[:, a0:a0 + 4, n * CW:(n + 1) * CW])
                    ts.append(xt_t)
                xts[n] = ts

            def gemm_mm_thunks(n):
                ps = pg.tile([128, 2, CW], f32, tag="g", name=f"g{n}")
                gps[n] = ps
                thunks = []
                for h in range(2):
                    for c in range(2):
                        for k in range(KI // 2):
                            ic = h * (KI // 2) + k
                            thunks.append(lambda h=h, c=c, k=k, ic=ic: mm(
                                ps[:, c, :], lhsT=wtiles[ic][:, c * 128:(c + 1) * 128],
                                rhs=xts[n][h][:, k, :],
                                start=(ic == 0), stop=False,
                                skip_group_check=True))
                for c in range(2):
                    thunks.append(lambda c=c: mm(
                        ps[:, c, :], lhsT=sb_bcombT[:, c, :], rhs=sb_ones,
                        start=False, stop=True, skip_group_check=True))
                return thunks

            def epi_thunks(n):
                sl = slice(n * CW, (n + 1) * CW)
                tsl = slice(n * TPC, (n + 1) * TPC)
                ps_h = pe.tile([128, CW], f32, tag="ph", name=f"ph{n}")
                ps_o = po.tile([O, CW], f32, tag="po", name=f"po{n}")
                return [
                    lambda: mm(ps_h, lhsT=sb_woutT[:, 0, :], rhs=etiles[n][:, 0, :],
                               start=True, stop=False),
                    lambda: mm(ps_h, lhsT=sb_woutT[:, 1, :], rhs=etiles[n][:, 1, :],
                               start=False, stop=True),
                    lambda: nc.scalar.activation(sb_hh[:, sl], ps_h, Relu,
                                                 bias=sb_bout, scale=1.0),
                    lambda: mm(ps_o, lhsT=sb_w2T, rhs=sb_hh[:, sl],
                               start=True, stop=True),
                    lambda: nc.vector.tensor_scalar_add(sb_o[:, sl], ps_o, sb_b2pp),
                ] + [
                    (lambda oo=oo: nc.sync.dma_start(
                        out=out_otb[oo:oo + 1, tsl, :],
                        in_=sb_o_r[oo:oo + 1, tsl, :])) for oo in range(O)
                ]

            def relu_step(ps, n, j):
                # One bias-free DVE op covers both halves (bias was folded into
                # the GEMM as a K=1 ones-matmul).
                sl = slice(j * BL, (j + 1) * BL)
                nc.vector.tensor_scalar_max(etiles[n][:, :, sl], ps[:, :, sl], 0.0)

            def recur_block(n, fillers, fillers2=()):
                ps = gps.pop(n)
                f2 = iter(fillers2)
                for j in range(TPC):
                    t = n * TPC + j
                    if t > 0:
                        pn, pj = (t - 1) // TPC, (t - 1) % TPC
                        prev = [etiles[pn][:, kc, pj * BL:(pj + 1) * BL]
                                for kc in range(2)]
                        sl = slice(j * BL, (j + 1) * BL)
                        for kc in range(2):
                            for c in range(2):
                                mm(ps[:, c, sl],
                                   lhsT=sb_wrecTr[:, kc, c * 128:(c + 1) * 128],
                                   rhs=prev[kc], start=False, stop=(kc == 1),
                                   skip_group_check=True)
                    th = next(fillers, None)
                    if th:
                        th()
                    if j % 7 == 3:
                        th2 = next(f2, None)
                        if th2:
                            th2()
                    relu_step(ps, n, j)
                for th in fillers:
                    th()
                for th in f2:
                    th()

            emit_dmas(0)
            for th in gemm_mm_thunks(0):
                th()
            for n in range(1, NCH):
                emit_dmas(n)
                recur_block(n - 1, iter(gemm_mm_thunks(n)),
                            epi_thunks(n - 2) if n >= 2 else ())
            recur_block(NCH - 1, iter(()), epi_thunks(NCH - 2))
            for th in epi_thunks(NCH - 1):
                th()

    if final:
        nc.compile()
    return nc


def _in_maps(x, W_in, b_in, W_rec, b_rec, W_out, b_out, W_out2, b_out2,
             gemm_dtype=GEMM_DTYPE):
    xnp = np.float16 if gemm_dtype == "float16" else np.float32
    maps = []
    for c in range(NCORES):
        xb = np.ascontiguousarray(
            x[c * BL:(c + 1) * BL].transpose(2, 1, 0).reshape(IN, NT).astype(xnp))
        maps.append({
            "xt": xb,
            "w_in": np.ascontiguousarray(W_in),
            "b_in": np.ascontiguousarray(b_in.reshape(S, 1)),
            "w_rec": np.ascontiguousarray(W_rec),
            "b_rec": np.ascontiguousarray(b_rec.reshape(S, 1)),
            "w_out": np.ascontiguousarray(W_out),
            "b_out": np.ascontiguousarray(b_out.reshape(H, 1)),
            "w_out2": np.ascontiguousarray(W_out2),
            "b_out2": np.ascontiguousarray(b_out2.reshape(O, 1)),
        })
    return maps


def kernel(x, W_in, b_in, W_rec, b_rec, W_out, b_out, W_out2, b_out2):
    from concourse.bass_utils import run_bass_kernel_spmd

    args = [np.asarray(a, dtype=np.float32) for a in
            (x, W_in, b_in, W_rec, b_rec, W_out, b_out, W_out2, b_out2)]
    nc = build_nc()
    res = run_bass_kernel_spmd(nc, _in_maps(*args), list(range(NCORES))).results
    out = np.empty((B, T, O), np.float32)
    for c in range(NCORES):
        out[c * BL:(c + 1) * BL] = res[c]["out"]
    return out


if __name__ == "__main__":
    rng = np.random.default_rng(0)
    ins = {
        "x": rng.standard_normal((B, T, IN), dtype=np.float32),
        "W_in": rng.standard_normal((S, IN), dtype=np.float32) * 0.01,
        "b_in": rng.standard_normal((S,), dtype=np.float32) * 0.01,
        "W_rec": rng.standard_normal((S, S), dtype=np.float32) * 0.05,
        "b_rec": rng.standard_normal((S,), dtype=np.float32) * 0.05,
        "W_out": rng.standard_normal((H, S), dtype=np.float32) * 0.05,
        "b_out": rng.standard_normal((H,), dtype=np.float32) * 0.05,
        "W_out2": rng.standard_normal((O, H), dtype=np.float32) * 0.05,
        "b_out2": rng.standard_normal((O,), dtype=np.float32) * 0.05,
    }
    print(kernel(**ins).shape)
